# revision 33
# baseline (speedup 1.0000x reference)
"""Trainium2 Bass kernel for CosineAttention:

    out = sigmoid((xn @ xn.T) @ x)   where xn = x / ||x_row||

Key algebraic optimization: reassociate (xn @ xn.T) @ x = xn @ (xn.T @ x).
G = xn.T @ x is [D, D] - the O(N^2 D) similarity matrix is never formed.

Sharding: rows of x across 8 cores. Each core:
  1. loads its [N/8, D] row block, computes row norms + normalized rows
  2. computes partial G'_c = xn_c.T @ x_c - (c/8)*I  (f32 PSUM accum)
  3. AllReduce across the 8 cores (fp16 payload)
  4. out_c = sigmoid(xn_c @ G' + c*xn_c)
The host concatenates the 8 row blocks.

G is symmetric, so only the lower-triangle blocks of the left
column-half (26 blocks packed to 832KB) and the lower-right quadrant
(512KB) are AllReduced; mirrored blocks are reconstructed on-chip by
PE-transposing blocks of the first AllReduce result. mm1 shrinks
accordingly (variable-width matmuls for the left half, upper-right
quadrant skipped entirely).

The c*xn correction for col-half 0 is seeded on the PE as one
accumulating matmul per output tile ((c*I).T @ xnr rowblock, issued in
the AR1 idle window); col-half 1 adds it on DVE so those matmuls stay
off the PE tail.

Observed TOPSP behavior: the first collective mesh begins only ~10us
after the LAST doorbell in the NEFF has fired, and meshes execute
serially. So both doorbells are pushed as early as the data
dependencies allow; a tiny warmup AllGather prepends the mesh train to
absorb first-collective ncfw setup (~10us cheaper than paying it in
AR1). Sum-of-squares runs on DVE (affine_mul_reduce) so ACT never
swaps activation tables; loads stream in 5 staggered chunks so mm1
starts at ~14us and paces the PE densely to the first doorbell.
"""

import numpy as np

import concourse.bass as bass  # noqa: F401
import concourse.mybir as mybir
import concourse.tile as tile
from concourse import bacc
from concourse.bass_utils import run_bass_kernel_spmd
from concourse.masks import make_identity

F32 = mybir.dt.float32
BF16 = mybir.dt.bfloat16
F16 = mybir.dt.float16
AFT = mybir.ActivationFunctionType

N, D = 8192, 1024
NCORES = 8
R = N // NCORES  # rows per core
P = 128
RT = R // P      # row tiles per core
KT = D // P      # contraction tiles (mm2) / G row tiles
FD = 512         # matmul moving free dim (one PSUM bank of f32)
NH = D // FD     # column halves
QT = KT // 2     # tiles per half (4)
GROUPS = [list(range(NCORES))]
DIAG_C = 256.0   # ~mean of diag(G); exact in fp16


def _emit_body(tc, xb, out, ctx):
    nc = tc.nc
    mm_dt = F16
    xb_t = xb.rearrange("(rt p) d -> rt p d", p=P)
    out_t = out.rearrange("(rt p) d -> rt p d", p=P)

    persist = ctx.enter_context(tc.tile_pool(name="persist", bufs=1))
    load = ctx.enter_context(tc.tile_pool(name="load", bufs=3))
    small = ctx.enter_context(tc.tile_pool(name="small", bufs=1))
    ostage = ctx.enter_context(tc.tile_pool(name="ostage", bufs=8))
    ps = ctx.enter_context(tc.tile_pool(name="ps", bufs=1, space="PSUM"))
    dram = ctx.enter_context(tc.tile_pool(name="dram", bufs=1, space="DRAM"))

    # ---- warmup collective: absorbs first-collective ncfw setup (the
    # first mesh otherwise pays ~10us extra). The TOPSP begins its
    # first mesh only after the LAST doorbell in the NEFF fires, so
    # this just prepends a cheap 8us mesh to the train while removing
    # AR1's setup penalty.
    w_in = dram.tile([P, 4], F32, tag="w_in")
    w_out = dram.tile([P * NCORES, 4], F32, tag="w_out", addr_space="Shared")
    nc.gpsimd.collective_compute(
        "AllGather", mybir.AluOpType.bypass, replica_groups=GROUPS,
        ins=[w_in.opt()], outs=[w_out.opt()],
    )

    # ---- phase 0: chunked loads, cast to fp16, norms ----
    # The row block streams in 5 chunks ({0},{1,2},{3,4},{5,6},{7}) so
    # the first tile lands early and completions stagger. Row sum of
    # squares runs on DVE (affine_mul_reduce), so ACT does only tiny
    # per-tile Rsqrts -- zero activation-table swaps. Fully streaming:
    # each tile's xn is ready ~1us after its chunk lands.
    CHUNKS = [(0, 1), (1, 3), (3, 5), (5, 7), (7, 8)]
    xfall = persist.tile([P, RT, D], F32, tag="xfall")
    xb_r = xb.rearrange("(rt p) d -> p rt d", p=P)
    for lo, hi in CHUNKS:
        nc.sync.dma_start(out=xfall[:, lo:hi, :], in_=xb_r[:, lo:hi, :])
    xbr, xnr = [], []
    ss_all = small.tile([P, RT], F32, tag="ss_all")
    nrm_all = small.tile([P, RT], F32, tag="nrm_all")
    rn_all = small.tile([P, RT], F32, tag="rn_all")
    for rt in range(RT):
        xf = xfall[:, rt, :]
        sq = load.tile([P, D], BF16, tag="sq")
        nc.vector.affine_mul_reduce(sq, ss_all[:, rt:rt + 1], xf, xf,
                                    1.0, 0.0)
        nc.scalar.sqrt(nrm_all[:, rt:rt + 1], ss_all[:, rt:rt + 1])
        nc.vector.reciprocal(rn_all[:, rt:rt + 1], nrm_all[:, rt:rt + 1])
        t_xbr = persist.tile([P, D], mm_dt, tag=f"xbr{rt}", name=f"xbr{rt}")
        nc.vector.tensor_copy(out=t_xbr, in_=xf)
        xbr.append(t_xbr)
        t_xnr = persist.tile([P, D], mm_dt, tag=f"xnr{rt}", name=f"xnr{rt}")
        nc.vector.tensor_scalar_mul(t_xnr, t_xbr, rn_all[:, rt:rt + 1])
        xnr.append(t_xnr)

    # identity / diag-shift constants (emitted after the loads so their
    # DVE/ACT setup doesn't delay the load-issue critical path)
    identb = persist.tile([P, P], mm_dt, tag="identb")
    make_identity(nc, identb)
    identc = persist.tile([P, P], mm_dt, tag="identc")
    nc.scalar.mul(identc, identb, DIAG_C)
    dsh = []
    for s in range(FD // P):
        t_dsh = persist.tile([P, FD], mm_dt, tag=f"dsh{s}", name=f"dsh{s}")
        nc.vector.memset(t_dsh, 0.0)
        nc.scalar.mul(t_dsh[:, s * P:(s + 1) * P], identb, -DIAG_C / NCORES)
        dsh.append(t_dsh)

    # ---- phase 1a: G' left half, lower-triangle blocks only ----
    # Row-block mt of cols 0:512 only needs block-cols j <= mt (the
    # upper blocks are mirrors of lower ones): variable-width matmuls,
    # payload packed to 26 blocks = 832KB.
    W = [min(mt + 1, QT) * P for mt in range(KT)]   # kept width per mt
    OFFS = [0]
    for mt in range(KT - 1):
        OFFS.append(OFFS[-1] + W[mt])
    TOT = OFFS[-1] + W[-1]                          # 3328 cols packed
    HALF = TOT // 2                                 # split point for DMAs
    g_in0 = dram.tile([P, TOT], mm_dt, tag="g_in0")
    g_out0 = dram.tile([P, TOT], mm_dt, tag="g_out0", addr_space="Shared")

    psg0 = [ps.tile([P, W[mt]], F32, tag=f"acc{mt}", name=f"psg0_{mt}")
            for mt in range(KT)]
    for rt in range(RT):
        for mt in range(KT):
            nc.tensor.matmul(
                psg0[mt],
                lhsT=xnr[rt][:, mt * P:(mt + 1) * P],
                rhs=xbr[rt][:, 0:W[mt]],
                start=(rt == 0),
                stop=(rt == RT - 1) and mt >= QT,
            )
    for mt in range(QT):
        # diag blocks live at mt 0..3 for the left column-half
        nc.tensor.matmul(psg0[mt], lhsT=identb, rhs=dsh[mt][:, :W[mt]],
                         start=False, stop=True)

    gA = persist.tile([P, TOT], mm_dt, tag="gA")
    for mt in range(KT):
        if mt < QT:
            nc.vector.tensor_copy(out=gA[:, OFFS[mt]:OFFS[mt] + W[mt]],
                                  in_=psg0[mt])
        else:
            nc.scalar.copy(out=gA[:, OFFS[mt]:OFFS[mt] + W[mt]],
                           in_=psg0[mt])
    nc.sync.dma_start(out=g_in0[:, :HALF], in_=gA[:, :HALF])
    nc.scalar.dma_start(out=g_in0[:, HALF:], in_=gA[:, HALF:])
    nc.gpsimd.collective_compute(
        "AllReduce", mybir.AluOpType.add, replica_groups=GROUPS,
        ins=[g_in0.opt()], outs=[g_out0.opt()],
    )

    # ---- phase 1b: G' lower-right quadrant rows/cols 512:1024 ----
    g_in1 = dram.tile([FD, FD], mm_dt, tag="g_in1")
    g_out1 = dram.tile([FD, FD], mm_dt, tag="g_out1", addr_space="Shared")
    g_in1_g = g_in1.rearrange("(q p) f -> p q f", p=P)
    g_out1_g = g_out1.rearrange("(q p) f -> p q f", p=P)

    psg1 = [ps.tile([P, FD], F32, tag=f"acc{QT + q}", name=f"psg1_{q}")
            for q in range(QT)]
    for rt in range(RT):
        for q in range(QT):
            nc.tensor.matmul(
                psg1[q],
                lhsT=xnr[rt][:, (QT + q) * P:(QT + q + 1) * P],
                rhs=xbr[rt][:, FD:],
                start=(rt == 0),
                stop=False,
            )
    for q in range(QT):
        nc.tensor.matmul(psg1[q], lhsT=identb, rhs=dsh[q],
                         start=False, stop=True)
    gB = persist.tile([P, QT, FD], mm_dt, tag="gB")
    for q in range(QT):
        if q % 2 == 0:
            nc.vector.tensor_copy(out=gB[:, q, :], in_=psg1[q])
        else:
            nc.scalar.copy(out=gB[:, q, :], in_=psg1[q])
    nc.sync.dma_start(out=g_in1_g[:, 0:2, :], in_=gB[:, 0:2, :])
    nc.scalar.dma_start(out=g_in1_g[:, 2:4, :], in_=gB[:, 2:4, :])
    nc.gpsimd.collective_compute(
        "AllReduce", mybir.AluOpType.add, replica_groups=GROUPS,
        ins=[g_in1.opt()], outs=[g_out1.opt()],
    )

    # ---- phase 1c (hidden in AR windows): cxn for col-half 1 ----
    # (col-half 0 gets c*xn via identc seed matmuls in the AR1 window;
    # col-half 1 uses DVE adds so those MMs stay off the PE tail)
    rc_all = small.tile([P, RT], F32, tag="rc_all")
    nc.scalar.mul(rc_all, rn_all, DIAG_C)
    cxn1 = []
    for rt in range(RT):
        t_cxn = persist.tile([P, FD], F32, tag=f"cxn{rt}", name=f"cxn{rt}")
        nc.vector.tensor_scalar_mul(t_cxn, xbr[rt][:, FD:],
                                    rc_all[:, rt:rt + 1])
        cxn1.append(t_cxn)

    # ---- phase 1c (hidden in AR windows): xnT transposes ----
    xnT = []
    for kt in range(KT):
        t_xnT = persist.tile([P, D], mm_dt, tag=f"xnT{kt}", name=f"xnT{kt}")
        for rt in range(RT):
            src = xnr[rt][:, kt * P:(kt + 1) * P]
            tpt = ps.tile([P, P], mm_dt, tag=f"acc{rt % 2}",
                          name=f"tp{kt}_{rt}")
            nc.tensor.transpose(tpt, src, identb)
            if rt % 2 == 0:
                nc.vector.tensor_copy(out=t_xnT[:, rt * P:(rt + 1) * P],
                                      in_=tpt)
            else:
                nc.scalar.copy(out=t_xnT[:, rt * P:(rt + 1) * P], in_=tpt)
        xnT.append(t_xnT)

    # h0's c*xn seeds for banks 2..7 run now, in the AR1 idle window
    # (banks 0/1 are still needed by the gq reconstruction transposes,
    # so their tiles are created and seeded after those)
    psz0 = [None] * RT
    for mt in range(2, RT):
        psz0[mt] = ps.tile([P, FD], F32, tag=f"acc{mt}",
                           name=f"psz0_{mt}")
        nc.tensor.matmul(psz0[mt], lhsT=identc, rhs=xnr[mt][:, 0:FD],
                         start=True, stop=False)

    # ---- phase 2: G loads. Emitted before mm2 so their ring slots sit
    # ahead of the output stores: each DMA's sem-wait releases the
    # moment its AllReduce ends. The packed payload loads in two halves
    # (second half first carries kt5..7, whose waves run first).
    grp = persist.tile([P, TOT], mm_dt, tag="grp")
    nc.scalar.dma_start(out=grp[:, HALF:], in_=g_out0[:, HALF:])
    nc.sync.dma_start(out=grp[:, :HALF], in_=g_out0[:, :HALF])

    # reassemble gr0[kt] (rows kt, cols 0:512): kt>=3 are direct slices
    # of the packed payload; kt<3 need their upper blocks mirrored from
    # block (j, kt) via PE transpose
    gq = [persist.tile([P, FD], mm_dt, tag=f"gq{i}", name=f"gq{i}")
          for i in range(3)]
    for i in range(3):
        if i % 2 == 0:
            nc.vector.tensor_copy(out=gq[i][:, :W[i]],
                                  in_=grp[:, OFFS[i]:OFFS[i] + W[i]])
        else:
            nc.scalar.copy(out=gq[i][:, :W[i]],
                           in_=grp[:, OFFS[i]:OFFS[i] + W[i]])
        for j in range(i + 1, QT):
            tpu = ps.tile([P, P], mm_dt, tag=f"acc{j % 2}",
                          name=f"tpu{i}_{j}")
            nc.tensor.transpose(
                tpu, grp[:, OFFS[j] + i * P:OFFS[j] + (i + 1) * P], identb)
            if j % 2 == 0:
                nc.vector.tensor_copy(out=gq[i][:, j * P:(j + 1) * P],
                                      in_=tpu)
            else:
                nc.scalar.copy(out=gq[i][:, j * P:(j + 1) * P], in_=tpu)
    gr0 = gq + [grp[:, OFFS[kt]:OFFS[kt] + FD] for kt in range(3, KT)]

    # remaining h0 seeds for banks 0/1 (gq transposes have released them)
    for mt in range(2):
        psz0[mt] = ps.tile([P, FD], F32, tag=f"acc{mt}",
                           name=f"psz0_{mt}")
        nc.tensor.matmul(psz0[mt], lhsT=identc, rhs=xnr[mt][:, 0:FD],
                         start=True, stop=False)

    # quadrant load for col-half 1 (sem-blocks the sync ring until AR2
    # ends, which also defers the h0 stores out of the AR2 mesh window)
    grq_t = persist.tile([P, QT, FD], mm_dt, tag="grq_t")
    nc.sync.dma_start(out=grq_t, in_=g_out1_g)
    grq = [grq_t[:, q, :] for q in range(QT)]

    def mm2_half(nh, gr, kt_order, psz=None, cxn=None):
        if psz is None:
            psz = [ps.tile([P, FD], F32, tag=f"acc{mt}",
                           name=f"psz{nh}_{mt}") for mt in range(RT)]
        for kt in kt_order[:-1]:
            for mt in range(RT):
                nc.tensor.matmul(
                    psz[mt],
                    lhsT=xnT[kt][:, mt * P:(mt + 1) * P],
                    rhs=gr[kt],
                    start=(kt == kt_order[0]) and cxn is not None,
                    stop=False,
                )
        for mt in range(RT):
            # last k-wave interleaved per tile so sigmoids/stores start
            # streaming while the remaining tiles finish on the PE
            kt = kt_order[-1]
            nc.tensor.matmul(
                psz[mt],
                lhsT=xnT[kt][:, mt * P:(mt + 1) * P],
                rhs=gr[kt],
                start=False, stop=True,
            )
            if cxn is not None:
                # c*xn via DVE add: keeps these 8 MMs off the PE tail
                nc.vector.tensor_add(psz[mt], psz[mt], cxn[mt])
            # sigmoid emits fp16 (halves the ACT write time; output is in
            # (0,1) so this costs ~3e-4 rel err); the SWDGE store casts
            # back to f32 on the way to DRAM
            ob = ostage.tile([P, FD], F16, tag="ob")
            nc.scalar.activation(out=ob, in_=psz[mt], func=AFT.Sigmoid)
            lo = nh * FD
            nc.gpsimd.dma_start(out=out_t[mt][:, lo:lo + FD], in_=ob)

    # descending kt: direct payload slices (kt7..3) stream first while
    # the gq reconstruction finishes
    mm2_half(0, gr0, kt_order=list(range(KT - 1, -1, -1)), psz=psz0)

    # upper-right quadrant G'[0:512, 512:1024] = blockwise PE transpose
    # of G'[512:1024, 0:512] (= gr0[4..7]); runs between the two mm2
    # halves, hidden under the h0 sigmoid/store drain
    grT = [persist.tile([P, FD], mm_dt, tag=f"grT{q}", name=f"grT{q}")
           for q in range(QT)]
    for q in range(QT):          # target row-block q (cols 512:1024)
        for b in range(QT):      # source row-block 4+b
            tpq = ps.tile([P, P], mm_dt, tag=f"acc{b % 2}",
                          name=f"tpq{q}_{b}")
            nc.tensor.transpose(tpq, gr0[QT + b][:, q * P:(q + 1) * P],
                                identb)
            if b % 2 == 0:
                nc.vector.tensor_copy(out=grT[q][:, b * P:(b + 1) * P],
                                      in_=tpq)
            else:
                nc.scalar.copy(out=grT[q][:, b * P:(b + 1) * P], in_=tpq)

    mm2_half(1, grT + grq, kt_order=list(range(KT)), cxn=cxn1)


def build():
    from contextlib import ExitStack

    nc = bacc.Bacc("TRN2", target_bir_lowering=False, debug=False,
                   num_devices=NCORES)
    xb = nc.dram_tensor("xb", [R, D], F32, kind="ExternalInput").ap()
    out = nc.dram_tensor("out", [R, D], F32, kind="ExternalOutput").ap()
    with tile.TileContext(nc) as tc:
        with ExitStack() as ctx:
            _emit_body(tc, xb, out, ctx)
    nc.compile()
    return nc


_NC_CACHE = {}


def _get_nc():
    if "nc" not in _NC_CACHE:
        _NC_CACHE["nc"] = build()
    return _NC_CACHE["nc"]


def kernel(x: np.ndarray) -> np.ndarray:
    x = np.asarray(x, dtype=np.float32)
    assert x.shape == (N, D), x.shape
    nc = _get_nc()
    in_maps = [{"xb": x[c * R:(c + 1) * R]} for c in range(NCORES)]
    res = run_bass_kernel_spmd(nc, in_maps, list(range(NCORES)))
    return np.concatenate([res.results[c]["out"] for c in range(NCORES)], axis=0)


# revision 35
# speedup vs baseline: 1.2059x; 1.2059x over previous
"""Trainium2 Bass kernel for CosineAttention:

    out = sigmoid((xn @ xn.T) @ x)   where xn = x / ||x_row||

Key algebraic optimization: reassociate (xn @ xn.T) @ x = xn @ (xn.T @ x).
G = xn.T @ x is [D, D] - the O(N^2 D) similarity matrix is never formed.

Sharding: rows of x across 8 cores. Each core:
  1. loads its [N/8, D] row block, computes row norms + normalized rows
  2. computes partial G'_c = xn_c.T @ x_c - (c/8)*I  (f32 PSUM accum)
  3. AllReduce across the 8 cores (fp16 payload)
  4. out_c = sigmoid(xn_c @ G' + c*xn_c)
The host concatenates the 8 row blocks.

G is symmetric, so only the lower-triangle blocks of the left
column-half (26 blocks packed to 832KB) and the lower-right quadrant
(512KB) are AllReduced; mirrored blocks are reconstructed on-chip by
PE-transposing blocks of the first AllReduce result. mm1 shrinks
accordingly (variable-width matmuls for the left half, upper-right
quadrant skipped entirely).

The c*xn correction for col-half 0 is seeded on the PE as one
accumulating matmul per output tile ((c*I).T @ xnr rowblock, issued in
the AR1 idle window); col-half 1 adds it on DVE so those matmuls stay
off the PE tail.

Observed TOPSP behavior: the first collective mesh begins only ~10us
after the LAST doorbell in the NEFF has fired, and meshes execute
serially. So both doorbells are pushed as early as the data
dependencies allow; a tiny warmup AllGather prepends the mesh train to
absorb first-collective ncfw setup (~10us cheaper than paying it in
AR1). Sum-of-squares runs on DVE (affine_mul_reduce) so ACT never
swaps activation tables; loads stream in 5 staggered chunks so mm1
starts at ~14us and paces the PE densely to the first doorbell.
"""

import numpy as np

import concourse.bass as bass  # noqa: F401
import concourse.mybir as mybir
import concourse.tile as tile
from concourse import bacc
from concourse.bass_utils import run_bass_kernel_spmd
from concourse.masks import make_identity

F32 = mybir.dt.float32
BF16 = mybir.dt.bfloat16
F16 = mybir.dt.float16
AFT = mybir.ActivationFunctionType

N, D = 8192, 1024
NCORES = 8
R = N // NCORES  # rows per core
P = 128
RT = R // P      # row tiles per core
KT = D // P      # contraction tiles (mm2) / G row tiles
FD = 512         # matmul moving free dim (one PSUM bank of f32)
NH = D // FD     # column halves
QT = KT // 2     # tiles per half (4)
GROUPS = [list(range(NCORES))]
DIAG_C = 256.0   # ~mean of diag(G); exact in fp16


def _emit_body(tc, xb, out, ctx):
    nc = tc.nc
    mm_dt = F16
    xb_t = xb.rearrange("(rt p) d -> rt p d", p=P)
    out_t = out.rearrange("(rt p) d -> rt p d", p=P)

    persist = ctx.enter_context(tc.tile_pool(name="persist", bufs=1))
    load = ctx.enter_context(tc.tile_pool(name="load", bufs=3))
    small = ctx.enter_context(tc.tile_pool(name="small", bufs=1))
    ostage = ctx.enter_context(tc.tile_pool(name="ostage", bufs=8))
    ps = ctx.enter_context(tc.tile_pool(name="ps", bufs=1, space="PSUM"))
    dram = ctx.enter_context(tc.tile_pool(name="dram", bufs=1, space="DRAM"))

    # ---- warmup collective: absorbs first-collective ncfw setup (the
    # first mesh otherwise pays ~10us extra). The TOPSP begins its
    # first mesh only after the LAST doorbell in the NEFF fires, so
    # this just prepends a cheap 8us mesh to the train while removing
    # AR1's setup penalty.
    w_in = dram.tile([P, 4], F32, tag="w_in")
    w_out = dram.tile([P * NCORES, 4], F32, tag="w_out", addr_space="Shared")
    nc.gpsimd.collective_compute(
        "AllGather", mybir.AluOpType.bypass, replica_groups=GROUPS,
        ins=[w_in.opt()], outs=[w_out.opt()],
    )

    # ---- phase 0: chunked loads, cast to fp16, norms ----
    # The row block streams in 5 chunks ({0},{1,2},{3,4},{5,6},{7}) so
    # the first tile lands early and completions stagger. Row sum of
    # squares runs on DVE (affine_mul_reduce), so ACT does only tiny
    # per-tile Rsqrts -- zero activation-table swaps. Fully streaming:
    # each tile's xn is ready ~1us after its chunk lands.
    CHUNKS = [(0, 1), (1, 2), (2, 4), (4, 6), (6, 8)]
    xfall = persist.tile([P, RT, D], F32, tag="xfall")
    xb_r = xb.rearrange("(rt p) d -> p rt d", p=P)
    # tiny dummy sqrt first so ACT loads the Sqrt table while the first
    # chunk is still in flight
    dumin = small.tile([P, 1], F32, tag="dumin")
    dumout = small.tile([P, 1], F32, tag="dumout")
    nc.vector.memset(dumin, 1.0)
    nc.scalar.sqrt(dumout, dumin)
    for lo, hi in CHUNKS:
        nc.sync.dma_start(out=xfall[:, lo:hi, :], in_=xb_r[:, lo:hi, :])
    xbr, xnr = [], []
    ss_all = small.tile([P, RT], F32, tag="ss_all")
    nrm_all = small.tile([P, RT], F32, tag="nrm_all")
    rn_all = small.tile([P, RT], F32, tag="rn_all")
    for rt in range(RT):
        xf = xfall[:, rt, :]
        sq = load.tile([P, D], BF16, tag="sq")
        nc.vector.affine_mul_reduce(sq, ss_all[:, rt:rt + 1], xf, xf,
                                    1.0, 0.0)
        nc.scalar.sqrt(nrm_all[:, rt:rt + 1], ss_all[:, rt:rt + 1])
        nc.vector.reciprocal(rn_all[:, rt:rt + 1], nrm_all[:, rt:rt + 1])
        t_xbr = persist.tile([P, D], mm_dt, tag=f"xbr{rt}", name=f"xbr{rt}")
        nc.vector.tensor_copy(out=t_xbr, in_=xf)
        xbr.append(t_xbr)
        t_xnr = persist.tile([P, D], mm_dt, tag=f"xnr{rt}", name=f"xnr{rt}")
        nc.vector.tensor_scalar_mul(t_xnr, t_xbr, rn_all[:, rt:rt + 1])
        xnr.append(t_xnr)

    # identity / diag-shift constants (emitted after the loads so their
    # DVE/ACT setup doesn't delay the load-issue critical path)
    identb = persist.tile([P, P], mm_dt, tag="identb")
    make_identity(nc, identb)
    identc = persist.tile([P, P], mm_dt, tag="identc")
    nc.scalar.mul(identc, identb, DIAG_C)
    dsh = []
    for s in range(FD // P):
        t_dsh = persist.tile([P, FD], mm_dt, tag=f"dsh{s}", name=f"dsh{s}")
        nc.vector.memset(t_dsh, 0.0)
        nc.scalar.mul(t_dsh[:, s * P:(s + 1) * P], identb, -DIAG_C / NCORES)
        dsh.append(t_dsh)

    # ---- phase 1a: G' left half, lower-triangle blocks only ----
    # Row-block mt of cols 0:512 only needs block-cols j <= mt (the
    # upper blocks are mirrors of lower ones): variable-width matmuls,
    # payload packed to 26 blocks = 832KB.
    W = [min(mt + 1, QT) * P for mt in range(KT)]   # kept width per mt
    OFFS = [0]
    for mt in range(KT - 1):
        OFFS.append(OFFS[-1] + W[mt])
    TOT = OFFS[-1] + W[-1]                          # 3328 cols packed
    HALF = TOT // 2                                 # split point for DMAs
    g_in0 = dram.tile([P, TOT], mm_dt, tag="g_in0")
    g_out0 = dram.tile([P, TOT], mm_dt, tag="g_out0", addr_space="Shared")

    psg0 = [ps.tile([P, W[mt]], F32, tag=f"acc{mt}", name=f"psg0_{mt}")
            for mt in range(KT)]
    for rt in range(RT):
        for mt in range(KT):
            nc.tensor.matmul(
                psg0[mt],
                lhsT=xnr[rt][:, mt * P:(mt + 1) * P],
                rhs=xbr[rt][:, 0:W[mt]],
                start=(rt == 0),
                stop=(rt == RT - 1) and mt >= QT,
            )
    for mt in range(QT):
        # diag blocks live at mt 0..3 for the left column-half
        nc.tensor.matmul(psg0[mt], lhsT=identb, rhs=dsh[mt][:, :W[mt]],
                         start=False, stop=True)

    gA = persist.tile([P, TOT], mm_dt, tag="gA")
    for mt in range(KT):
        if mt < QT:
            nc.vector.tensor_copy(out=gA[:, OFFS[mt]:OFFS[mt] + W[mt]],
                                  in_=psg0[mt])
        else:
            nc.scalar.copy(out=gA[:, OFFS[mt]:OFFS[mt] + W[mt]],
                           in_=psg0[mt])
    nc.sync.dma_start(out=g_in0[:, :HALF], in_=gA[:, :HALF])
    nc.scalar.dma_start(out=g_in0[:, HALF:], in_=gA[:, HALF:])
    nc.gpsimd.collective_compute(
        "AllReduce", mybir.AluOpType.add, replica_groups=GROUPS,
        ins=[g_in0.opt()], outs=[g_out0.opt()],
    )

    # ---- phase 1b: G' lower-right quadrant rows/cols 512:1024 ----
    g_in1 = dram.tile([FD, FD], mm_dt, tag="g_in1")
    g_out1 = dram.tile([FD, FD], mm_dt, tag="g_out1", addr_space="Shared")
    g_in1_g = g_in1.rearrange("(q p) f -> p q f", p=P)
    g_out1_g = g_out1.rearrange("(q p) f -> p q f", p=P)

    psg1 = [ps.tile([P, FD], F32, tag=f"acc{QT + q}", name=f"psg1_{q}")
            for q in range(QT)]
    for rt in range(RT):
        for q in range(QT):
            nc.tensor.matmul(
                psg1[q],
                lhsT=xnr[rt][:, (QT + q) * P:(QT + q + 1) * P],
                rhs=xbr[rt][:, FD:],
                start=(rt == 0),
                stop=False,
            )
    for q in range(QT):
        nc.tensor.matmul(psg1[q], lhsT=identb, rhs=dsh[q],
                         start=False, stop=True)
    gB = persist.tile([P, QT, FD], mm_dt, tag="gB")
    for q in range(QT):
        if q % 2 == 0:
            nc.vector.tensor_copy(out=gB[:, q, :], in_=psg1[q])
        else:
            nc.scalar.copy(out=gB[:, q, :], in_=psg1[q])
    nc.sync.dma_start(out=g_in1_g[:, 0:2, :], in_=gB[:, 0:2, :])
    nc.scalar.dma_start(out=g_in1_g[:, 2:4, :], in_=gB[:, 2:4, :])
    nc.gpsimd.collective_compute(
        "AllReduce", mybir.AluOpType.add, replica_groups=GROUPS,
        ins=[g_in1.opt()], outs=[g_out1.opt()],
    )

    # ---- phase 1c (hidden in AR windows): cxn for col-half 1 ----
    # (col-half 0 gets c*xn via identc seed matmuls in the AR1 window;
    # col-half 1 uses DVE adds so those MMs stay off the PE tail)
    rc_all = small.tile([P, RT], F32, tag="rc_all")
    nc.scalar.mul(rc_all, rn_all, DIAG_C)
    cxn1 = []
    for rt in range(RT):
        t_cxn = persist.tile([P, FD], F32, tag=f"cxn{rt}", name=f"cxn{rt}")
        nc.vector.tensor_scalar_mul(t_cxn, xbr[rt][:, FD:],
                                    rc_all[:, rt:rt + 1])
        cxn1.append(t_cxn)

    # ---- phase 1c (hidden in AR windows): xnT transposes ----
    xnT = []
    for kt in range(KT):
        t_xnT = persist.tile([P, D], mm_dt, tag=f"xnT{kt}", name=f"xnT{kt}")
        for rt in range(RT):
            src = xnr[rt][:, kt * P:(kt + 1) * P]
            tpt = ps.tile([P, P], mm_dt, tag=f"acc{rt % 2}",
                          name=f"tp{kt}_{rt}")
            nc.tensor.transpose(tpt, src, identb)
            if rt % 2 == 0:
                nc.vector.tensor_copy(out=t_xnT[:, rt * P:(rt + 1) * P],
                                      in_=tpt)
            else:
                nc.scalar.copy(out=t_xnT[:, rt * P:(rt + 1) * P], in_=tpt)
        xnT.append(t_xnT)

    # h0's c*xn seeds for banks 2..7 run now, in the AR1 idle window
    # (banks 0/1 are still needed by the gq reconstruction transposes,
    # so their tiles are created and seeded after those)
    psz0 = [None] * RT
    for mt in range(2, RT):
        psz0[mt] = ps.tile([P, FD], F32, tag=f"acc{mt}",
                           name=f"psz0_{mt}")
        nc.tensor.matmul(psz0[mt], lhsT=identc, rhs=xnr[mt][:, 0:FD],
                         start=True, stop=False)

    # ---- phase 2: G loads. Emitted before mm2 so their ring slots sit
    # ahead of the output stores: each DMA's sem-wait releases the
    # moment its AllReduce ends. The packed payload loads in two halves
    # (second half first carries kt5..7, whose waves run first).
    grp = persist.tile([P, TOT], mm_dt, tag="grp")
    nc.scalar.dma_start(out=grp[:, HALF:], in_=g_out0[:, HALF:])
    nc.sync.dma_start(out=grp[:, :HALF], in_=g_out0[:, :HALF])

    # reassemble gr0[kt] (rows kt, cols 0:512): kt>=3 are direct slices
    # of the packed payload; kt<3 need their upper blocks mirrored from
    # block (j, kt) via PE transpose
    gq = [persist.tile([P, FD], mm_dt, tag=f"gq{i}", name=f"gq{i}")
          for i in range(3)]
    for i in range(3):
        if i % 2 == 0:
            nc.vector.tensor_copy(out=gq[i][:, :W[i]],
                                  in_=grp[:, OFFS[i]:OFFS[i] + W[i]])
        else:
            nc.scalar.copy(out=gq[i][:, :W[i]],
                           in_=grp[:, OFFS[i]:OFFS[i] + W[i]])
        for j in range(i + 1, QT):
            tpu = ps.tile([P, P], mm_dt, tag=f"acc{j % 2}",
                          name=f"tpu{i}_{j}")
            nc.tensor.transpose(
                tpu, grp[:, OFFS[j] + i * P:OFFS[j] + (i + 1) * P], identb)
            if j % 2 == 0:
                nc.vector.tensor_copy(out=gq[i][:, j * P:(j + 1) * P],
                                      in_=tpu)
            else:
                nc.scalar.copy(out=gq[i][:, j * P:(j + 1) * P], in_=tpu)
    gr0 = gq + [grp[:, OFFS[kt]:OFFS[kt] + FD] for kt in range(3, KT)]

    # remaining h0 seeds for banks 0/1 (gq transposes have released them)
    for mt in range(2):
        psz0[mt] = ps.tile([P, FD], F32, tag=f"acc{mt}",
                           name=f"psz0_{mt}")
        nc.tensor.matmul(psz0[mt], lhsT=identc, rhs=xnr[mt][:, 0:FD],
                         start=True, stop=False)

    # quadrant load for col-half 1 (sem-blocks the sync ring until AR2
    # ends, which also defers the h0 stores out of the AR2 mesh window)
    grq_t = persist.tile([P, QT, FD], mm_dt, tag="grq_t")
    nc.sync.dma_start(out=grq_t, in_=g_out1_g)
    grq = [grq_t[:, q, :] for q in range(QT)]

    def mm2_half(nh, gr, kt_order, psz=None, cxn=None):
        if psz is None:
            psz = [ps.tile([P, FD], F32, tag=f"acc{mt}",
                           name=f"psz{nh}_{mt}") for mt in range(RT)]
        for kt in kt_order[:-1]:
            for mt in range(RT):
                nc.tensor.matmul(
                    psz[mt],
                    lhsT=xnT[kt][:, mt * P:(mt + 1) * P],
                    rhs=gr[kt],
                    start=(kt == kt_order[0]) and cxn is not None,
                    stop=False,
                )
        for mt in range(RT):
            # last k-wave interleaved per tile so sigmoids/stores start
            # streaming while the remaining tiles finish on the PE
            kt = kt_order[-1]
            nc.tensor.matmul(
                psz[mt],
                lhsT=xnT[kt][:, mt * P:(mt + 1) * P],
                rhs=gr[kt],
                start=False, stop=True,
            )
            if cxn is not None:
                # c*xn via DVE add: keeps these 8 MMs off the PE tail
                nc.vector.tensor_add(psz[mt], psz[mt], cxn[mt])
            # sigmoid emits fp16 (halves the ACT write time; output is in
            # (0,1) so this costs ~3e-4 rel err); the SWDGE store casts
            # back to f32 on the way to DRAM. In the final drain the
            # single SWDGE queue serializes, so h1 alternates f32
            # sigmoids + HWDGE-ring stores to spread across 3 queues.
            lo = nh * FD
            if nh == 1 and mt % 2 == 1:
                ob32 = ostage.tile([P, FD], F32, tag="ob32")
                nc.scalar.activation(out=ob32, in_=psz[mt],
                                     func=AFT.Sigmoid)
                eng = nc.sync if mt % 4 == 1 else nc.scalar
                eng.dma_start(out=out_t[mt][:, lo:lo + FD], in_=ob32)
            else:
                ob = ostage.tile([P, FD], F16, tag="ob")
                nc.scalar.activation(out=ob, in_=psz[mt], func=AFT.Sigmoid)
                nc.gpsimd.dma_start(out=out_t[mt][:, lo:lo + FD], in_=ob)

    # descending kt: direct payload slices (kt7..3) stream first while
    # the gq reconstruction finishes
    mm2_half(0, gr0, kt_order=list(range(KT - 1, -1, -1)), psz=psz0)

    # upper-right quadrant G'[0:512, 512:1024] = blockwise PE transpose
    # of G'[512:1024, 0:512] (= gr0[4..7]); runs between the two mm2
    # halves, hidden under the h0 sigmoid/store drain
    grT = [persist.tile([P, FD], mm_dt, tag=f"grT{q}", name=f"grT{q}")
           for q in range(QT)]
    for q in range(QT):          # target row-block q (cols 512:1024)
        for b in range(QT):      # source row-block 4+b
            tpq = ps.tile([P, P], mm_dt, tag=f"acc{b % 2}",
                          name=f"tpq{q}_{b}")
            nc.tensor.transpose(tpq, gr0[QT + b][:, q * P:(q + 1) * P],
                                identb)
            if b % 2 == 0:
                nc.vector.tensor_copy(out=grT[q][:, b * P:(b + 1) * P],
                                      in_=tpq)
            else:
                nc.scalar.copy(out=grT[q][:, b * P:(b + 1) * P], in_=tpq)

    mm2_half(1, grT + grq, kt_order=list(range(KT)), cxn=cxn1)


def build():
    from contextlib import ExitStack

    nc = bacc.Bacc("TRN2", target_bir_lowering=False, debug=False,
                   num_devices=NCORES)
    xb = nc.dram_tensor("xb", [R, D], F32, kind="ExternalInput").ap()
    out = nc.dram_tensor("out", [R, D], F32, kind="ExternalOutput").ap()
    with tile.TileContext(nc) as tc:
        with ExitStack() as ctx:
            _emit_body(tc, xb, out, ctx)
    nc.compile()
    return nc


_NC_CACHE = {}


def _get_nc():
    if "nc" not in _NC_CACHE:
        _NC_CACHE["nc"] = build()
    return _NC_CACHE["nc"]


def kernel(x: np.ndarray) -> np.ndarray:
    x = np.asarray(x, dtype=np.float32)
    assert x.shape == (N, D), x.shape
    nc = _get_nc()
    in_maps = [{"xb": x[c * R:(c + 1) * R]} for c in range(NCORES)]
    res = run_bass_kernel_spmd(nc, in_maps, list(range(NCORES)))
    return np.concatenate([res.results[c]["out"] for c in range(NCORES)], axis=0)
